# revision 1
# baseline (speedup 1.0000x reference)
"""Multi-head attention block (qkv -> attention -> o_net -> residual+LN) on
8 Trainium2 NeuronCores.

Problem (hardcoded): B=2, T=2048, D=1024, H=16, dh=64, fp32 I/O.
Reference quirk: the (B,H,T,dh) attention buffer is viewed as (H,B,T,dh)
before the output projection, i.e. output batch b2 / head-slot h2 takes the
attention output of original (b, h) with 16*b + h == 2*h2 + b2.

Sharding: tokens split along T only -> core c owns positions
[c*256, (c+1)*256) of BOTH batches (512 tokens).  Every core computes qkv for
its tokens, AllGathers K and V (Q stays local), runs attention for ALL 32
(b,h) pairs restricted to its query positions, applies the o_net with the
permutation above, then residual + layernorm on its tokens.  All device
addresses are identical across cores (pure SPMD); only the data differs.
"""
import sys
sys.path.insert(0, "/opt/trn_rl_repo")
import contextlib
import numpy as np
import ml_dtypes

import concourse.bass as bass
from concourse import bacc
import concourse.mybir as mybir
import concourse.tile as tile
from concourse.bass_utils import run_bass_kernel_spmd

BF16 = mybir.dt.bfloat16
F32 = mybir.dt.float32
nbf16 = ml_dtypes.bfloat16

N_CORES = 8
B, T, D = 2, 2048, 1024
H, DH = 16, 64
TC = T // N_CORES          # 256 query positions per core
NTOK = B * TC              # 512 tokens per core (both batches)
LN_EPS = 1e-5

KT_ROWS = 1024             # k-channel rows in the AG buffer
V_W = H * (DH + 1)         # 1040: V row width with ones column per head
KT_SZ = KT_ROWS * NTOK     # 524288 elems
V_SZ = NTOK * V_W          # 532480 elems
AGS = KT_SZ + V_SZ         # per-rank AG elems (1,056,768)

_prog_cache = {}

# bf16 scores in PSUM: halves the exp instruction count (bigger ACT ops).
SCORE_BF16 = False


def _build_program(reps=1, score_bf16=SCORE_BF16):
    """reps>1 repeats the attention + o_net phases (timing-only builds)."""
    nc = bacc.Bacc("TRN2", num_devices=N_CORES)

    # ---- per-core inputs (host pre-tiled / pre-transposed, bf16) ----
    inpT = nc.dram_tensor("inpT", [128, 8, NTOK], BF16, kind="ExternalInput")
    inp_res = nc.dram_tensor("inp_res", [NTOK, D], F32, kind="ExternalInput")
    wqkT = nc.dram_tensor("wqkT", [128, 8, 2048], BF16, kind="ExternalInput")
    wvT = nc.dram_tensor("wvT", [128, 8, 1024], BF16, kind="ExternalInput")
    woT = nc.dram_tensor("woT", [128, 8, 1024], BF16, kind="ExternalInput")
    b_qk = nc.dram_tensor("b_qk", [1, 2048], BF16, kind="ExternalInput")
    b_v = nc.dram_tensor("b_v", [1, 1024], BF16, kind="ExternalInput")
    onesd = nc.dram_tensor("onesd", [1, NTOK], BF16, kind="ExternalInput")
    gamma = nc.dram_tensor("gamma", [1, D], F32, kind="ExternalInput")
    beta = nc.dram_tensor("beta", [1, D], F32, kind="ExternalInput")

    out = nc.dram_tensor("out", [NTOK, D], F32, kind="ExternalOutput")

    def bcast_rows(src_row_ap, nrows):
        return bass.AP(tensor=src_row_ap.tensor, offset=src_row_ap.offset,
                       ap=[[0, nrows]] + src_row_ap.ap[1:])

    # scores-block layout: pair A fills the first half of the tile's banks,
    # pair B the second (concurrent row-packed matmuls must target different
    # PSUM banks, so each pair-half must be >= 1 full bank -> block >= 2 kt).
    # Empirically fastest: 8 blocks of 2 kt, double-buffered:
    #   pscr2 (2 banks x 2 bufs) + poA (1) + poB (1) = 6 of 8 banks.
    import os as _os
    _blk = _os.environ.get("KBLOCKS", "2,2,2,2,2,2,2,2")
    SCORE_DT, BLOCKS = F32, [int(x) for x in _blk.split(",")]
    assert sum(BLOCKS) == 16
    PSB = int(_os.environ.get("KPSB", "2"))
    EVAC = _os.environ.get("KEVAC", "1") == "1"
    KTDB, PTB, NRMB = [int(x) for x in _os.environ.get("KBUFS", "2,2,3").split(",")]
    PROBE = _os.environ.get("KPROBE", "")  # timing-only ablations

    with tile.TileContext(nc) as tc:
        with contextlib.ExitStack() as ctx:
            dram = ctx.enter_context(tc.tile_pool(name="dram", bufs=1, space="DRAM"))
            dram_sc = ctx.enter_context(tc.tile_pool(name="dram_sc", bufs=4, space="DRAM"))
            cst = ctx.enter_context(tc.tile_pool(name="cst", bufs=1))

            agin = dram.tile([AGS], BF16)
            agout_k = dram.tile([N_CORES, KT_SZ], BF16, addr_space="Shared")
            VH = V_SZ // 2      # per-batch half of the V region (rows 0-255 / 256-511)
            agout_v0 = dram.tile([N_CORES, VH], BF16, addr_space="Shared")
            agout_v1 = dram.tile([N_CORES, VH], BF16, addr_space="Shared")

            ones_sb = cst.tile([1, NTOK], BF16)
            nc.sync.dma_start(out=ones_sb[:], in_=onesd[:])
            bqk_sb = cst.tile([1, 2048], BF16)
            nc.sync.dma_start(out=bqk_sb[:], in_=b_qk[:])
            bv_sb = cst.tile([1, 1024], BF16)
            nc.sync.dma_start(out=bv_sb[:], in_=b_v[:])

            qT_sb = cst.tile([128, 8, NTOK], BF16)   # Q^T kept on-chip

            # ---------------- phase 1: qkv projection -----------------
            with tc.tile_pool(name="projw", bufs=1) as projw, \
                 tc.tile_pool(name="pstage", bufs=3) as pstage, \
                 tc.tile_pool(name="psproj", bufs=4, space="PSUM") as psproj:
                wqk_sb = projw.tile([128, 8, 2048], BF16)
                wv_sb = projw.tile([128, 8, 1024], BF16)
                inpT_sb = projw.tile([128, 8, NTOK], BF16)
                for dt in range(8):
                    nc.sync.dma_start(out=inpT_sb[:, dt, :], in_=inpT[:, dt, :])
                    nc.sync.dma_start(out=wqk_sb[:, dt, :], in_=wqkT[:, dt, :])
                    nc.sync.dma_start(out=wv_sb[:, dt, :], in_=wvT[:, dt, :])

                # --- K^T: channels 1024..2047 of qkv, layout [1024, 512] ---
                for cc in range(8):
                    pp = psproj.tile([128, NTOK], F32, tag="pp")
                    nc.tensor.matmul(out=pp[:], lhsT=bqk_sb[0:1, 1024 + cc * 128: 1024 + (cc + 1) * 128],
                                     rhs=ones_sb[:], start=True, stop=False)
                    for dt in range(8):
                        nc.tensor.matmul(out=pp[:], lhsT=wqk_sb[:, dt, 1024 + cc * 128: 1024 + (cc + 1) * 128],
                                         rhs=inpT_sb[:, dt, :], start=False, stop=(dt == 7))
                    ks = pstage.tile([128, NTOK], BF16, tag="ks")
                    nc.vector.tensor_copy(out=ks[:], in_=pp[:])
                    dst = agin[cc * 128 * NTOK: (cc + 1) * 128 * NTOK]
                    nc.sync.dma_start(out=dst.rearrange("(p f) -> p f", p=128), in_=ks[:])

                # --- AllGather of K^T as soon as it's staged ---
                nc.gpsimd.collective_compute(
                    "AllGather", mybir.AluOpType.bypass,
                    replica_groups=[list(range(N_CORES))],
                    ins=[agin[0:KT_SZ]], outs=[agout_k[:]],
                )


                # --- Q^T: channels 0..1023, kept in SBUF (overlaps AG) ---
                for cc in range(8):
                    pp = psproj.tile([128, NTOK], F32, tag="pp")
                    nc.tensor.matmul(out=pp[:], lhsT=bqk_sb[0:1, cc * 128: (cc + 1) * 128],
                                     rhs=ones_sb[:], start=True, stop=False)
                    for dt in range(8):
                        nc.tensor.matmul(out=pp[:], lhsT=wqk_sb[:, dt, cc * 128: (cc + 1) * 128],
                                         rhs=inpT_sb[:, dt, :], start=False, stop=(dt == 7))
                    nc.vector.tensor_copy(out=qT_sb[:, cc, :], in_=pp[:])

                # --- V: [512 tok, 1040] with ones col per head ---
                for tch in range(4):
                    vs = pstage.tile([128, H, DH + 1], BF16, tag="vs")
                    nc.vector.memset(vs[:, :, DH: DH + 1], 1.0)
                    for ncv in range(2):
                        pv = psproj.tile([128, 512], F32, tag="pv")
                        nc.tensor.matmul(out=pv[:], lhsT=ones_sb[0:1, 0:128],
                                         rhs=bv_sb[0:1, ncv * 512: (ncv + 1) * 512],
                                         start=True, stop=False)
                        for dt in range(8):
                            nc.tensor.matmul(out=pv[:], lhsT=inpT_sb[:, dt, tch * 128: (tch + 1) * 128],
                                             rhs=wv_sb[:, dt, ncv * 512: (ncv + 1) * 512],
                                             start=False, stop=(dt == 7))
                        nc.vector.tensor_copy(
                            out=vs[:, ncv * 8: (ncv + 1) * 8, 0:DH],
                            in_=pv[:].rearrange("p (h d) -> p h d", d=DH))
                    dst = agin[KT_SZ + tch * 128 * V_W: KT_SZ + (tch + 1) * 128 * V_W]
                    nc.sync.dma_start(out=dst.rearrange("(p f) -> p f", p=128), in_=vs[:])

                # --- AllGather of V', split per batch: the b'=0 half lands
                # first so early duos start while the b'=1 half is in flight ---
                nc.gpsimd.collective_compute(
                    "AllGather", mybir.AluOpType.bypass,
                    replica_groups=[list(range(N_CORES))],
                    ins=[agin[KT_SZ: KT_SZ + VH]], outs=[agout_v0[:]],
                )
                nc.gpsimd.collective_compute(
                    "AllGather", mybir.AluOpType.bypass,
                    replica_groups=[list(range(N_CORES))],
                    ins=[agin[KT_SZ + VH:]], outs=[agout_v1[:]],
                )

            # ---------------- phase 3: attention over 16 duos ----------
            # duo d: pairs (2d, 2d+1); b' = d//8, h'_A = 2d%16 (even), h'_B = h'_A+1
            # output: duo d -> slot d; pair A -> out batch 0, pair B -> batch 1
            attnv = [cst.tile([128, 8, TC], BF16, name=f"attnv{b2}") for b2 in range(2)]
            if reps == 0:   # timing-only build: keep o_net inputs defined
                nc.vector.memset(attnv[0][:], 0.0)
                nc.vector.memset(attnv[1][:], 0.0)

            for _rep in range(reps):
              with tc.tile_pool(name="vfull", bufs=2) as vfull_pool, \
                 tc.tile_pool(name="att", bufs=KTDB) as att, \
                 tc.tile_pool(name="pt", bufs=3) as ptp, \
                 tc.tile_pool(name="nrm", bufs=NRMB) as nrm, \
                 tc.tile_pool(name="pss", bufs=3, space="PSUM") as pss, \
                 tc.tile_pool(name="pso", bufs=1, space="PSUM") as pso:

                vfull = None
                for d in range(16):
                    bp = d // 8
                    hA = (2 * d) % 16
                    row_off = 128 * (d % 8)

                    if d % 8 == 0:
                        # load V' for batch bp: [128, kt, head, 65]
                        vfull = vfull_pool.tile([128, 16, H, DH + 1], BF16, tag="vf")
                        for kt in range(16):
                            r = kt // 2
                            agv = agout_v0 if bp == 0 else agout_v1
                            off = r * VH + ((kt % 2) * 128) * V_W
                            src = bass.AP(tensor=agv.tensor, offset=agv.offset + off,
                                          ap=[[V_W, 128], [1, V_W]])
                            nc.sync.dma_start(
                                out=vfull[:, kt, :, :].rearrange("p h d -> p (h d)"), in_=src)

                    # K^T duo [128 rows, 8 ranks, 256]
                    ktd = att.tile([128, 8, TC], BF16, tag="ktd")
                    src = bass.AP(tensor=agout_k.tensor,
                                  offset=agout_k.offset + row_off * NTOK + bp * TC,
                                  ap=[[NTOK, 128], [KT_SZ, 8], [1, TC]])
                    nc.sync.dma_start(out=ktd[:], in_=src)

                    qd = qT_sb[:, d % 8, bp * TC: (bp + 1) * TC]   # [128, 256]

                    poA = pso.tile([65, TC], F32, tag="poA")
                    poB = pso.tile([65, TC], F32, tag="poB")
                    kt0 = 0
                    for bw in BLOCKS:   # kt-tiles per scores block
                        half = bw * TC  # elems per pair-half (bank-aligned)
                        pscr = pss.tile([128, 2 * half], SCORE_DT,
                                        tag=f"pscr{bw}", bufs=PSB)
                        for j2 in range(bw):
                            kt = kt0 + j2
                            nc.tensor.matmul(out=pscr[:, j2 * TC: (j2 + 1) * TC],
                                             lhsT=ktd[0:64, kt // 2, (kt % 2) * 128: (kt % 2 + 1) * 128],
                                             rhs=qd[0:64, :],
                                             start=True, stop=True, tile_position=(0, 0))
                            nc.tensor.matmul(out=pscr[:, half + j2 * TC: half + (j2 + 1) * TC],
                                             lhsT=ktd[64:128, kt // 2, (kt % 2) * 128: (kt % 2 + 1) * 128],
                                             rhs=qd[64:128, :],
                                             start=True, stop=True, tile_position=(64, 0))
                        pt = ptp.tile([128, 2 * half], BF16, tag=f"pt{bw}", bufs=PTB)
                        nc.scalar.activation(out=pt[:], in_=pscr[:],
                                             func=mybir.ActivationFunctionType.Exp, scale=0.125)
                        if PROBE == "exp2x":
                            ptx = ptp.tile([128, 2 * half], BF16, tag=f"ptx{bw}", bufs=PTB)
                            nc.scalar.activation(out=ptx[:], in_=pscr[:],
                                                 func=mybir.ActivationFunctionType.Exp, scale=0.125)
                        for j2 in range(bw):
                            kt = kt0 + j2
                            if PROBE == "mm22x":
                                poX = pso.tile([65, TC], F32, tag="poX", bufs=2)
                                nc.tensor.matmul(out=poX[:], lhsT=vfull[:, kt, hA, :],
                                                 rhs=pt[:, j2 * TC: (j2 + 1) * TC],
                                                 start=True, stop=True)
                                nc.tensor.matmul(out=poX[:], lhsT=vfull[:, kt, hA + 1, :],
                                                 rhs=pt[:, half + j2 * TC: half + (j2 + 1) * TC],
                                                 start=False, stop=True, skip_group_check=True)
                            nc.tensor.matmul(out=poA[:], lhsT=vfull[:, kt, hA, :],
                                             rhs=pt[:, j2 * TC: (j2 + 1) * TC],
                                             start=(kt == 0), stop=(kt == 15))
                            nc.tensor.matmul(out=poB[:], lhsT=vfull[:, kt, hA + 1, :],
                                             rhs=pt[:, half + j2 * TC: half + (j2 + 1) * TC],
                                             start=(kt == 0), stop=(kt == 15))
                        kt0 += bw

                    # evacuate O' to SBUF promptly so the PSUM banks free up
                    # for the next duo (the normalize chain below has a DRAM
                    # round-trip we must keep off the PE critical path)
                    if EVAC:
                        oA = nrm.tile([65, TC], F32, tag="oA")
                        nc.vector.tensor_copy(out=oA[:], in_=poA[:])
                        oB = nrm.tile([65, TC], F32, tag="oB")
                        nc.vector.tensor_copy(out=oB[:], in_=poB[:])
                    else:
                        oA, oB = poA, poB

                    # normalize: rec = 1/denominator, broadcast via DRAM bounce
                    recA = nrm.tile([1, TC], F32, tag="recA")
                    nc.vector.reciprocal(out=recA[:], in_=oA[64:65, :])
                    recB = nrm.tile([1, TC], F32, tag="recB")
                    nc.vector.reciprocal(out=recB[:], in_=oB[64:65, :])
                    rec_d = dram_sc.tile([2, TC], F32, tag="rec_d")
                    nc.sync.dma_start(out=rec_d[0:1, :], in_=recA[:])
                    nc.sync.dma_start(out=rec_d[1:2, :], in_=recB[:])
                    rbA = nrm.tile([64, TC], F32, tag="rbA")
                    nc.gpsimd.dma_start(out=rbA[:], in_=bcast_rows(rec_d[0:1, :], 64))
                    rbB = nrm.tile([64, TC], F32, tag="rbB")
                    nc.gpsimd.dma_start(out=rbB[:], in_=bcast_rows(rec_d[1:2, :], 64))
                    jt, rhalf = d // 2, (d % 2) * 64
                    nc.vector.tensor_tensor(out=attnv[0][rhalf: rhalf + 64, jt, :],
                                            in0=oA[0:64, :], in1=rbA[:],
                                            op=mybir.AluOpType.mult)
                    nc.vector.tensor_tensor(out=attnv[1][rhalf: rhalf + 64, jt, :],
                                            in0=oB[0:64, :], in1=rbB[:],
                                            op=mybir.AluOpType.mult)

            # ---------------- phase 4: o_net + residual + layernorm ----
            with tc.tile_pool(name="fin", bufs=2) as fin, \
                 tc.tile_pool(name="finc", bufs=1) as finc, \
                 tc.tile_pool(name="psf", bufs=4, space="PSUM") as psf:
                wo_sb = finc.tile([128, 8, 1024], BF16)
                nc.sync.dma_start(out=wo_sb[:], in_=woT[:])
                res_sb = finc.tile([128, 4, D], F32)
                nc.sync.dma_start(out=res_sb[:],
                                  in_=inp_res.rearrange("(c p) d -> p c d", p=128))
                gb_sb = finc.tile([128, D], F32)
                nc.gpsimd.dma_start(out=gb_sb[:], in_=bcast_rows(gamma[0:1, :], 128))
                bb_sb = finc.tile([128, D], F32)
                nc.gpsimd.dma_start(out=bb_sb[:], in_=bcast_rows(beta[0:1, :], 128))
                eps_sb = finc.tile([128, 1], F32)
                nc.vector.memset(eps_sb[:], LN_EPS)

                for b2 in range(2):
                    for tch in range(2):
                        chunk = b2 * 2 + tch
                        x = fin.tile([128, D], F32, tag="x")
                        for nn_ in range(2):
                            po = psf.tile([128, 512], F32, tag="po")
                            for jt in range(8):
                                nc.tensor.matmul(out=po[:],
                                                 lhsT=attnv[b2][:, jt, tch * 128: (tch + 1) * 128],
                                                 rhs=wo_sb[:, jt, nn_ * 512: (nn_ + 1) * 512],
                                                 start=(jt == 0), stop=(jt == 7))
                            nc.vector.tensor_tensor(out=x[:, nn_ * 512: (nn_ + 1) * 512],
                                                    in0=po[:],
                                                    in1=res_sb[:, chunk, nn_ * 512: (nn_ + 1) * 512],
                                                    op=mybir.AluOpType.add)
                        stats = fin.tile([128, 2, 6], F32, tag="stats")
                        for s2 in range(2):
                            nc.vector.bn_stats(out=stats[:, s2, :], in_=x[:, s2 * 512: (s2 + 1) * 512])
                        mv = fin.tile([128, 2], F32, tag="mv")
                        nc.vector.bn_aggr(out=mv[:], in_=stats[:])
                        sd = fin.tile([128, 1], F32, tag="sd")
                        nc.scalar.activation(out=sd[:], in_=mv[:, 1:2],
                                             func=mybir.ActivationFunctionType.Sqrt,
                                             bias=eps_sb[:], scale=1.0)
                        rstd = fin.tile([128, 1], F32, tag="rstd")
                        nc.vector.reciprocal(out=rstd[:], in_=sd[:])
                        y = fin.tile([128, D], F32, tag="y")
                        nc.vector.tensor_scalar(out=y[:], in0=x[:],
                                                scalar1=mv[:, 0:1], scalar2=rstd[:],
                                                op0=mybir.AluOpType.subtract,
                                                op1=mybir.AluOpType.mult)
                        yg = fin.tile([128, D], F32, tag="yg")
                        nc.vector.tensor_tensor(out=yg[:], in0=y[:], in1=gb_sb[:],
                                                op=mybir.AluOpType.mult)
                        yb = fin.tile([128, D], F32, tag="yb")
                        nc.vector.tensor_tensor(out=yb[:], in0=yg[:], in1=bb_sb[:],
                                                op=mybir.AluOpType.add)
                        nc.sync.dma_start(
                            out=out[chunk * 128: (chunk + 1) * 128, :], in_=yb[:])

    nc.finalize()
    return nc


def _get_program(reps=1, score_bf16=None):
    import os as _os
    if score_bf16 is None:
        score_bf16 = SCORE_BF16
    key = ("nc", reps, score_bf16, _os.environ.get("KBLOCKS", ""), _os.environ.get("KPSB", ""), _os.environ.get("KEVAC", ""), _os.environ.get("KPROBE", ""), _os.environ.get("KBUFS", ""))
    if key not in _prog_cache:
        _prog_cache[key] = _build_program(reps, score_bf16)
    return _prog_cache[key]


def _prep_inputs(inp, W_qkv, b_qkv, W_o, gamma, beta):
    """Build the 8 per-core input dicts (host-side, all free)."""
    f32 = np.float32
    inp = np.asarray(inp, f32)
    W_qkv = np.asarray(W_qkv, f32)
    b_qkv = np.asarray(b_qkv, f32)
    W_o = np.asarray(W_o, f32)
    gamma = np.asarray(gamma, f32).reshape(1, D)
    beta = np.asarray(beta, f32).reshape(1, D)

    wqkT = np.ascontiguousarray(
        W_qkv[0:2048, :].T.reshape(8, 128, 2048).transpose(1, 0, 2)).astype(nbf16)
    wvT = np.ascontiguousarray(
        W_qkv[2048:3072, :].T.reshape(8, 128, 1024).transpose(1, 0, 2)).astype(nbf16)
    woT = np.ascontiguousarray(
        W_o.T.reshape(8, 128, 1024).transpose(1, 0, 2)).astype(nbf16)
    b_qk = b_qkv[0:2048].reshape(1, 2048).astype(nbf16)
    b_v = b_qkv[2048:3072].reshape(1, 1024).astype(nbf16)
    ones = np.ones((1, NTOK), nbf16)

    in_maps = []
    for c in range(N_CORES):
        sl = slice(c * TC, (c + 1) * TC)
        x = np.concatenate([inp[0, sl, :], inp[1, sl, :]], axis=0)  # [512, 1024]
        inpT = np.ascontiguousarray(
            x.T.reshape(8, 128, NTOK).transpose(1, 0, 2)).astype(nbf16)
        in_maps.append({
            "inpT": inpT,
            "inp_res": np.ascontiguousarray(x),
            "wqkT": wqkT, "wvT": wvT, "woT": woT,
            "b_qk": b_qk, "b_v": b_v, "onesd": ones,
            "gamma": gamma, "beta": beta,
        })
    return in_maps


def _assemble(results):
    out = np.empty((B, T, D), np.float32)
    for c in range(N_CORES):
        o = results[c]["out"]
        sl = slice(c * TC, (c + 1) * TC)
        out[0, sl, :] = o[0:TC, :]
        out[1, sl, :] = o[TC:NTOK, :]
    return out


def kernel(inp, W_qkv, b_qkv, W_o, gamma, beta):
    nc = _get_program()
    in_maps = _prep_inputs(inp, W_qkv, b_qkv, W_o, gamma, beta)
    res = run_bass_kernel_spmd(nc, in_maps, core_ids=list(range(N_CORES)))
    return _assemble(res.results)


if __name__ == "__main__":
    rng = np.random.RandomState(0)
    inp = rng.randn(B, T, D).astype(np.float32)
    W_qkv = (rng.randn(3 * H * DH, D) * D ** -0.5).astype(np.float32)
    b_qkv = (rng.randn(3 * H * DH) * 0.02).astype(np.float32)
    W_o = (rng.randn(D, H * DH) * (H * DH) ** -0.5).astype(np.float32)
    gamma = np.ones(D, np.float32)
    beta = np.zeros(D, np.float32)
    out = kernel(inp=inp, W_qkv=W_qkv, b_qkv=b_qkv, W_o=W_o, gamma=gamma, beta=beta)
    print("out", out.shape, out.dtype, np.abs(out).mean())



# revision 5
# speedup vs baseline: 2.2397x; 2.2397x over previous
"""Multi-head attention block (qkv -> attention -> o_net -> residual+LN) on
8 Trainium2 NeuronCores.

Problem (hardcoded): B=2, T=2048, D=1024, H=16, dh=64, fp32 I/O.
Reference quirk: the (B,H,T,dh) attention buffer is viewed as (H,B,T,dh)
before the output projection: out[b2, t, 64*h2:64*h2+64] takes the
attention output of original (b, h) with 16*b + h == 2*h2 + b2.

Sharding (head tensor-parallel, no K/V gather):
  core c owns batch b = c//4 and head quad hq = c%4 (heads 4hq..4hq+3).
  It computes Q/K/V for those 4 heads over ALL 2048 tokens of its batch
  locally (same flops as token sharding), runs full attention for its 4
  (b,h) units with K/V resident in SBUF, then reshards attn_vec from
  head-sharded to token-sharded via two small AllToAlls (one per local
  head pair; each moves 64 chans x 2048 tok bf16 = 0.5 MB per rank).
  After the A2A, core c holds all 1024 attn channels for output tokens
  (b2=c//4, t in [512*(c%4), 512*(c%4)+512)) -- by the view quirk, the
  A2A block from src s lands exactly at channel block [128s, 128s+128)
  (pair 0 -> low 64, pair 1 -> high 64).  o_net + residual + LN run on
  that 512-token shard.  Pure SPMD; all addresses identical across cores.
"""
import sys
sys.path.insert(0, "/opt/trn_rl_repo")
import contextlib
import numpy as np
import ml_dtypes

import concourse.bass as bass
from concourse import bacc
import concourse.mybir as mybir
import concourse.tile as tile
from concourse.bass_utils import run_bass_kernel_spmd

BF16 = mybir.dt.bfloat16
F32 = mybir.dt.float32
nbf16 = ml_dtypes.bfloat16

N_CORES = 8
B, T, D = 2, 2048, 1024
H, DH = 16, 64
TOK = T                    # tokens per core's batch
OTOK = 512                 # output token shard per core
LN_EPS = 1e-5

_prog_cache = {}


def _build_program(reps=1):
    """reps>1 repeats the attention+A2A phase (timing-only builds)."""
    assert reps >= 1
    nc = bacc.Bacc("TRN2", num_devices=N_CORES)

    # ---- per-core inputs (host pre-tiled / pre-transposed) ----
    inpT = nc.dram_tensor("inpT", [128, 8, TOK], BF16, kind="ExternalInput")
    wqT = nc.dram_tensor("wqT", [128, 8, 256], BF16, kind="ExternalInput")
    wkT = nc.dram_tensor("wkT", [128, 8, 256], BF16, kind="ExternalInput")
    wvT = nc.dram_tensor("wvT", [128, 8, 256], BF16, kind="ExternalInput")
    bq_col = nc.dram_tensor("bq_col", [128, 2], F32, kind="ExternalInput")
    bk_col = nc.dram_tensor("bk_col", [128, 2], F32, kind="ExternalInput")
    bv_row = nc.dram_tensor("bv_row", [1, 256], F32, kind="ExternalInput")
    woT = nc.dram_tensor("woT", [128, 8, 1024], BF16, kind="ExternalInput")
    inp_res = nc.dram_tensor("inp_res", [OTOK, D], F32, kind="ExternalInput")
    gamma = nc.dram_tensor("gamma", [1, D], F32, kind="ExternalInput")
    beta = nc.dram_tensor("beta", [1, D], F32, kind="ExternalInput")

    out = nc.dram_tensor("out", [OTOK, D], F32, kind="ExternalOutput")

    def bcast_rows(src_row_ap, nrows):
        return bass.AP(tensor=src_row_ap.tensor, offset=src_row_ap.offset,
                       ap=[[0, nrows]] + src_row_ap.ap[1:])

    with tile.TileContext(nc) as tc:
        with contextlib.ExitStack() as ctx:
            dram = ctx.enter_context(tc.tile_pool(name="dram", bufs=1, space="DRAM"))
            dram_sc = ctx.enter_context(tc.tile_pool(name="dram_sc", bufs=4, space="DRAM"))
            cst = ctx.enter_context(tc.tile_pool(name="cst", bufs=1))

            # A2A buffers: block j of a2a_in[p] = attn^T of local head
            # (2p + j//4) for tokens [512*(j%4), 512*(j%4)+512), as [64, 512].
            a2a_in = [dram.tile([8, 64, 512], BF16, name=f"a2a_in{p}")
                      for p in range(2)]
            a2a_out = [dram.tile([8, 64, 512], BF16, name=f"a2a_out{p}")
                       for p in range(2)]

            # persistent SBUF: Q^T, K^T (two head-pair tiles), V (+ones col)
            qT_sb = cst.tile([128, 2, TOK], BF16)
            kT_sb = cst.tile([128, 2, TOK], BF16)
            v_sb = cst.tile([128, 16, 4, DH + 1], BF16)

            # ---------------- phase 1: qkv projection -----------------
            with tc.tile_pool(name="projw", bufs=1) as projw, \
                 tc.tile_pool(name="psproj", bufs=3, space="PSUM") as psproj:
                wk_sb = projw.tile([128, 8, 256], BF16)
                nc.sync.dma_start(out=wk_sb[:], in_=wkT[:])
                inpT_sb = projw.tile([128, 8, TOK], BF16)
                for dt in range(8):
                    nc.sync.dma_start(out=inpT_sb[:, dt, :], in_=inpT[:, dt, :])
                wv_sb = projw.tile([128, 8, 256], BF16)
                nc.sync.dma_start(out=wv_sb[:], in_=wvT[:])
                wq_sb = projw.tile([128, 8, 256], BF16)
                nc.sync.dma_start(out=wq_sb[:], in_=wqT[:])
                bk_sb = projw.tile([128, 2], F32)
                nc.sync.dma_start(out=bk_sb[:], in_=bk_col[:])
                bq_sb = projw.tile([128, 2], F32)
                nc.sync.dma_start(out=bq_sb[:], in_=bq_col[:])
                bv_sb = projw.tile([128, 256], F32)
                nc.gpsimd.dma_start(out=bv_sb[:], in_=bcast_rows(bv_row[0:1, :], 128))

                nc.vector.memset(v_sb[:, :, :, DH:DH + 1], 1.0)

                # K^T: [128 chans (2 heads) x 2 tiles, 2048 tok]
                for p in range(2):
                    for ch in range(4):
                        pk = psproj.tile([128, 512], F32, tag="pp")
                        for dt in range(8):
                            nc.tensor.matmul(
                                out=pk[:],
                                lhsT=wk_sb[:, dt, p * 128:(p + 1) * 128],
                                rhs=inpT_sb[:, dt, ch * 512:(ch + 1) * 512],
                                start=(dt == 0), stop=(dt == 7))
                        nc.vector.tensor_scalar_add(
                            out=kT_sb[:, p, ch * 512:(ch + 1) * 512], in0=pk[:],
                            scalar1=bk_sb[:, p:p + 1])

                # V: [128 tok-block x 16, 4 heads, 65] (col 64 = ones)
                for tb in range(16):
                    pv = psproj.tile([128, 256], F32, tag="pv")
                    for dt in range(8):
                        nc.tensor.matmul(
                            out=pv[:],
                            lhsT=inpT_sb[:, dt, tb * 128:(tb + 1) * 128],
                            rhs=wv_sb[:, dt, :],
                            start=(dt == 0), stop=(dt == 7))
                    nc.vector.tensor_tensor(
                        out=v_sb[:, tb, :, 0:DH],
                        in0=pv[:].rearrange("p (h d) -> p h d", d=DH),
                        in1=bv_sb[:].rearrange("p (h d) -> p h d", d=DH),
                        op=mybir.AluOpType.add)

                # Q^T
                for p in range(2):
                    for ch in range(4):
                        pq = psproj.tile([128, 512], F32, tag="pp")
                        for dt in range(8):
                            nc.tensor.matmul(
                                out=pq[:],
                                lhsT=wq_sb[:, dt, p * 128:(p + 1) * 128],
                                rhs=inpT_sb[:, dt, ch * 512:(ch + 1) * 512],
                                start=(dt == 0), stop=(dt == 7))
                        nc.vector.tensor_scalar_add(
                            out=qT_sb[:, p, ch * 512:(ch + 1) * 512], in0=pq[:],
                            scalar1=bq_sb[:, p:p + 1])

            # ---------------- phase 2: attention + A2A -----------------
            for _rep in range(reps):
              with tc.tile_pool(name="pss", bufs=2, space="PSUM") as pss, \
                   tc.tile_pool(name="pso", bufs=2, space="PSUM") as pso, \
                   tc.tile_pool(name="ptp", bufs=3) as ptp, \
                   tc.tile_pool(name="nrm", bufs=3) as nrm:
                for p in range(2):
                    for qc in range(4):
                        q0 = qc * 512
                        poA = pso.tile([DH + 1, 512], F32, tag="poA")
                        poB = pso.tile([DH + 1, 512], F32, tag="poB")
                        for kb in range(16):
                            # scores: row-packed pair (concurrent K=64 matmuls)
                            pscr = pss.tile([128, 1024], F32, tag="pscr")
                            nc.tensor.matmul(
                                out=pscr[:, 0:512],
                                lhsT=kT_sb[0:64, p, kb * 128:(kb + 1) * 128],
                                rhs=qT_sb[0:64, p, q0:q0 + 512],
                                start=True, stop=True, tile_position=(0, 0))
                            nc.tensor.matmul(
                                out=pscr[:, 512:1024],
                                lhsT=kT_sb[64:128, p, kb * 128:(kb + 1) * 128],
                                rhs=qT_sb[64:128, p, q0:q0 + 512],
                                start=True, stop=True, tile_position=(64, 0))
                            pt = ptp.tile([128, 1024], BF16, tag="pt")
                            nc.scalar.activation(
                                out=pt[:], in_=pscr[:],
                                func=mybir.ActivationFunctionType.Exp, scale=0.125)
                            nc.tensor.matmul(out=poA[:],
                                             lhsT=v_sb[:, kb, 2 * p, :],
                                             rhs=pt[:, 0:512],
                                             start=(kb == 0), stop=(kb == 15))
                            nc.tensor.matmul(out=poB[:],
                                             lhsT=v_sb[:, kb, 2 * p + 1, :],
                                             rhs=pt[:, 512:1024],
                                             start=(kb == 0), stop=(kb == 15))

                        # normalize: evacuate, rec = 1/den (row 64), broadcast
                        # via DRAM bounce, scale, stage A2A blocks
                        oA = nrm.tile([DH + 1, 512], F32, tag="oA")
                        nc.vector.tensor_copy(out=oA[:], in_=poA[:])
                        oB = nrm.tile([DH + 1, 512], F32, tag="oB")
                        nc.vector.tensor_copy(out=oB[:], in_=poB[:])
                        recA = nrm.tile([1, 512], F32, tag="recA")
                        nc.vector.reciprocal(out=recA[:], in_=oA[DH:DH + 1, :])
                        recB = nrm.tile([1, 512], F32, tag="recB")
                        nc.vector.reciprocal(out=recB[:], in_=oB[DH:DH + 1, :])
                        rec_d = dram_sc.tile([2, 512], F32, tag="rec_d")
                        nc.sync.dma_start(out=rec_d[0:1, :], in_=recA[:])
                        nc.sync.dma_start(out=rec_d[1:2, :], in_=recB[:])
                        rbA = nrm.tile([DH, 512], F32, tag="rbA")
                        nc.gpsimd.dma_start(out=rbA[:], in_=bcast_rows(rec_d[0:1, :], DH))
                        rbB = nrm.tile([DH, 512], F32, tag="rbB")
                        nc.gpsimd.dma_start(out=rbB[:], in_=bcast_rows(rec_d[1:2, :], DH))
                        avA = nrm.tile([DH, 512], BF16, tag="avA")
                        nc.vector.tensor_tensor(out=avA[:], in0=oA[0:DH, :],
                                                in1=rbA[:], op=mybir.AluOpType.mult)
                        avB = nrm.tile([DH, 512], BF16, tag="avB")
                        nc.vector.tensor_tensor(out=avB[:], in0=oB[0:DH, :],
                                                in1=rbB[:], op=mybir.AluOpType.mult)
                        # head 2p (even parity) -> output batch 0 dests (j=qc);
                        # head 2p+1 -> batch 1 dests (j=4+qc)
                        nc.sync.dma_start(out=a2a_in[p][qc], in_=avA[:])
                        nc.sync.dma_start(out=a2a_in[p][4 + qc], in_=avB[:])

                    nc.gpsimd.collective_compute(
                        "AllToAll", mybir.AluOpType.bypass,
                        replica_groups=[list(range(N_CORES))],
                        ins=[a2a_in[p][:]], outs=[a2a_out[p][:]],
                    )

            # ---------------- phase 3: o_net + residual + layernorm ----
            with tc.tile_pool(name="fin", bufs=2) as fin, \
                 tc.tile_pool(name="finc", bufs=1) as finc, \
                 tc.tile_pool(name="psf", bufs=4, space="PSUM") as psf:
                wo_sb = finc.tile([128, 8, 1024], BF16)
                nc.sync.dma_start(out=wo_sb[:], in_=woT[:])
                res_sb = finc.tile([128, 4, D], F32)
                nc.sync.dma_start(out=res_sb[:],
                                  in_=inp_res.rearrange("(c p) d -> p c d", p=128))
                gb_sb = finc.tile([128, D], F32)
                nc.gpsimd.dma_start(out=gb_sb[:], in_=bcast_rows(gamma[0:1, :], 128))
                bb_sb = finc.tile([128, D], F32)
                nc.gpsimd.dma_start(out=bb_sb[:], in_=bcast_rows(beta[0:1, :], 128))
                eps_sb = finc.tile([128, 1], F32)
                nc.vector.memset(eps_sb[:], LN_EPS)

                # attn channels for my token shard: ic block cblk (128 wide)
                # = [A2A#0 block cblk (64) | A2A#1 block cblk (64)]
                icb_sb = finc.tile([128, 8, 512], BF16)
                for cblk in range(8):
                    nc.sync.dma_start(out=icb_sb[0:64, cblk, :], in_=a2a_out[0][cblk])
                    nc.sync.dma_start(out=icb_sb[64:128, cblk, :], in_=a2a_out[1][cblk])

                for tch in range(4):
                    x = fin.tile([128, D], F32, tag="x")
                    for nn_ in range(2):
                        po = psf.tile([128, 512], F32, tag="po")
                        for cblk in range(8):
                            nc.tensor.matmul(
                                out=po[:],
                                lhsT=icb_sb[:, cblk, tch * 128:(tch + 1) * 128],
                                rhs=wo_sb[:, cblk, nn_ * 512:(nn_ + 1) * 512],
                                start=(cblk == 0), stop=(cblk == 7))
                        nc.vector.tensor_tensor(out=x[:, nn_ * 512:(nn_ + 1) * 512],
                                                in0=po[:],
                                                in1=res_sb[:, tch, nn_ * 512:(nn_ + 1) * 512],
                                                op=mybir.AluOpType.add)
                    stats = fin.tile([128, 2, 6], F32, tag="stats")
                    for s2 in range(2):
                        nc.vector.bn_stats(out=stats[:, s2, :], in_=x[:, s2 * 512:(s2 + 1) * 512])
                    mv = fin.tile([128, 2], F32, tag="mv")
                    nc.vector.bn_aggr(out=mv[:], in_=stats[:])
                    sd = fin.tile([128, 1], F32, tag="sd")
                    nc.scalar.activation(out=sd[:], in_=mv[:, 1:2],
                                         func=mybir.ActivationFunctionType.Sqrt,
                                         bias=eps_sb[:], scale=1.0)
                    rstd = fin.tile([128, 1], F32, tag="rstd")
                    nc.vector.reciprocal(out=rstd[:], in_=sd[:])
                    y = fin.tile([128, D], F32, tag="y")
                    nc.vector.tensor_scalar(out=y[:], in0=x[:],
                                            scalar1=mv[:, 0:1], scalar2=rstd[:],
                                            op0=mybir.AluOpType.subtract,
                                            op1=mybir.AluOpType.mult)
                    yg = fin.tile([128, D], F32, tag="yg")
                    nc.vector.tensor_tensor(out=yg[:], in0=y[:], in1=gb_sb[:],
                                            op=mybir.AluOpType.mult)
                    yb = fin.tile([128, D], F32, tag="yb")
                    nc.vector.tensor_tensor(out=yb[:], in0=yg[:], in1=bb_sb[:],
                                            op=mybir.AluOpType.add)
                    nc.sync.dma_start(
                        out=out[tch * 128:(tch + 1) * 128, :], in_=yb[:])

    nc.finalize()
    return nc


def _get_program(reps=1):
    if reps not in _prog_cache:
        _prog_cache[reps] = _build_program(reps)
    return _prog_cache[reps]


def _prep_inputs(inp, W_qkv, b_qkv, W_o, gamma, beta):
    """Build the 8 per-core input dicts (host-side, all free)."""
    f32 = np.float32
    inp = np.asarray(inp, f32)
    W_qkv = np.asarray(W_qkv, f32)
    b_qkv = np.asarray(b_qkv, f32)
    W_o = np.asarray(W_o, f32)
    gamma = np.asarray(gamma, f32).reshape(1, D)
    beta = np.asarray(beta, f32).reshape(1, D)

    woT = np.ascontiguousarray(
        W_o.T.reshape(8, 128, 1024).transpose(1, 0, 2)).astype(nbf16)

    in_maps = []
    for c in range(N_CORES):
        b, qd = c // 4, c % 4
        rq = slice(256 * qd, 256 * qd + 256)
        rk = slice(1024 + 256 * qd, 1024 + 256 * qd + 256)
        rv = slice(2048 + 256 * qd, 2048 + 256 * qd + 256)

        def wtile(rows):
            return np.ascontiguousarray(
                W_qkv[rows].T.reshape(8, 128, 256).transpose(1, 0, 2)).astype(nbf16)

        x = inp[b]                                   # [2048, 1024]
        inpT_ = np.ascontiguousarray(
            x.T.reshape(8, 128, TOK).transpose(1, 0, 2)).astype(nbf16)
        in_maps.append({
            "inpT": inpT_,
            "wqT": wtile(rq), "wkT": wtile(rk), "wvT": wtile(rv),
            "bq_col": np.ascontiguousarray(b_qkv[rq].reshape(2, 128).T),
            "bk_col": np.ascontiguousarray(b_qkv[rk].reshape(2, 128).T),
            "bv_row": b_qkv[rv].reshape(1, 256).copy(),
            "woT": woT,
            "inp_res": np.ascontiguousarray(inp[b, 512 * qd: 512 * qd + 512]),
            "gamma": gamma, "beta": beta,
        })
    return in_maps


def _assemble(results):
    out = np.empty((B, T, D), np.float32)
    for c in range(N_CORES):
        out[c // 4, 512 * (c % 4): 512 * (c % 4) + 512, :] = results[c]["out"]
    return out


def kernel(inp, W_qkv, b_qkv, W_o, gamma, beta):
    nc = _get_program()
    in_maps = _prep_inputs(inp, W_qkv, b_qkv, W_o, gamma, beta)
    res = run_bass_kernel_spmd(nc, in_maps, core_ids=list(range(N_CORES)))
    return _assemble(res.results)


if __name__ == "__main__":
    rng = np.random.RandomState(0)
    inp = rng.randn(B, T, D).astype(np.float32)
    W_qkv = (rng.randn(3 * H * DH, D) * D ** -0.5).astype(np.float32)
    b_qkv = (rng.randn(3 * H * DH) * 0.02).astype(np.float32)
    W_o = (rng.randn(D, H * DH) * (H * DH) ** -0.5).astype(np.float32)
    gamma = np.ones(D, np.float32)
    beta = np.zeros(D, np.float32)
    out = kernel(inp=inp, W_qkv=W_qkv, b_qkv=b_qkv, W_o=W_o, gamma=gamma, beta=beta)
    print("out", out.shape, out.dtype, np.abs(out).mean())
